# revision 20
# baseline (speedup 1.0000x reference)
"""Trainium2 Bass kernel for CustomPointScatter.

Reference computation:
    pillar_feat = point_features.mean(axis=1)            # [N, C]
    out = zeros([B, C, H, W]); out[b, :, y, x] = pillar_feat

Sharding: each of the 8 cores owns one output region (b, y_half) of shape
[C, H/2, W].  The host partitions pillars by destination region, pads every
group to a common multiple-of-256 size, and hands each core its pillars plus
per-pillar destination row offsets.  On device the output region is laid out
position-major as [H/2 * W (+pad), C] so each pillar is a single contiguous
256 B row write, done with an indirect (scatter) DMA.  ExternalOutput DRAM is
delivered zero-initialised by the runtime, so only occupied rows are written.
The host reassembles the regions and transposes to [B, C, H, W].

Device pipeline per super-tile (SUP blocks of 128 pillars):
  1. HWDGE DMA loads points 0:16 of each pillar row.
  2. SWDGE DMA with accum_op=add folds points 16:32 on top (2:1 DMA fold).
  3. DVE halving adds finish the point-axis sum (16 -> 1).
  4. ACT scales by 1/32.
  5. Indirect DMA scatters each pillar's 256 B feature row.
"""

import numpy as np

import concourse.bacc as bacc
import concourse.bass as bass
import concourse.mybir as mybir
import concourse.tile as tile
from concourse.bass_utils import run_bass_kernel_spmd

B, H, W = 4, 512, 512
N_PILLARS, N_POINTS, C = 40000, 32, 64
N_CORES = 8
P = 128
HALF = H // 2            # 256 rows of the BEV grid per core
REGION_ROWS = HALF * W   # 131072 positions per core
PAD_ROWS = P             # dump rows for padded (inactive) pillars
OUT_ROWS = REGION_ROWS + PAD_ROWS
SUP = 2                  # pillar blocks (of 128) per super-tile
NBANKS = 4               # independent output tensors to break scatter WAW chains


def build_nc(nmax, n_points=N_POINTS, c=C, out_rows=OUT_ROWS, mode="fold2",
             sup=SUP, bufs=8, nbanks=NBANKS):
    T = nmax // P          # pillar blocks
    D = n_points * c       # full row: 2048 floats
    HD = D // 2            # half row after the 2:1 DMA fold
    nc = bacc.Bacc("TRN2", target_bir_lowering=False)
    pf = nc.dram_tensor("pf", [nmax, D], mybir.dt.float32, kind="ExternalInput")
    offs = nc.dram_tensor("offs", [P, T], mybir.dt.int32, kind="ExternalInput")
    out = nc.dram_tensor("out", [out_rows, c], mybir.dt.float32, kind="ExternalOutput")
    banks = [out] + [
        nc.dram_tensor(f"out{k}", [out_rows, c], mybir.dt.float32,
                       kind="ExternalOutput")
        for k in range(1, nbanks)
    ] if mode == "banks" else None
    with tile.TileContext(nc) as tc:
        with (
            tc.tile_pool(name="io", bufs=bufs) as io_pool,
            tc.tile_pool(name="misc", bufs=1) as misc,
        ):
            offs_sb = misc.tile([P, T], mybir.dt.int32)
            nc.sync.dma_start(out=offs_sb[:], in_=offs[:])
            if mode == "batch":
                # pillar j = t*sup*128 + p*sup + blk -> partition p, block blk
                # (sup consecutive DRAM rows per partition: one contiguous
                # 32KB descriptor span per partition per load)
                assert T % sup == 0
                for t in range(T // sup):
                    rows = slice(t * sup * P, (t + 1) * sup * P)
                    sb = io_pool.tile([P, sup * D], mybir.dt.float32, tag="sb")
                    nc.sync.dma_start(
                        out=sb[:],
                        in_=pf[rows, :].rearrange("(p blk) w -> p (blk w)", p=P),
                    )
                    v = sb[:].rearrange("p (blk w) -> p blk w", w=D)
                    w = D
                    while w > c:
                        w //= 2
                        nc.vector.tensor_add(
                            out=v[:, :, :w], in0=v[:, :, :w], in1=v[:, :, w:2 * w]
                        )
                    nc.scalar.mul(out=v[:, :, :c], in_=v[:, :, :c], mul=1.0 / n_points)
                    nc.gpsimd.indirect_dma_start(
                        out=out[:],
                        out_offset=bass.IndirectOffsetOnAxis(
                            ap=offs_sb[:, t * sup:(t + 1) * sup], axis=0
                        ),
                        in_=v[:, :, :c],
                        in_offset=None,
                    )
            elif mode == "fold2":
                assert T % sup == 0
                for t in range(T // sup):
                    rows = slice(t * sup * P, (t + 1) * sup * P)
                    sb = io_pool.tile([P, sup * HD], mybir.dt.float32, tag="sb")
                    v = sb[:].rearrange("p (blk w) -> p blk w", w=HD)
                    nc.sync.dma_start(
                        out=v,
                        in_=pf[rows, :HD].rearrange("(blk p) w -> p blk w", p=P),
                    )
                    nc.gpsimd.dma_start(
                        out=v,
                        in_=pf[rows, HD:].rearrange("(blk p) w -> p blk w", p=P),
                        accum_op=mybir.AluOpType.add,
                    )
                    w = HD
                    while w > c:
                        w //= 2
                        nc.vector.tensor_add(
                            out=v[:, :, :w], in0=v[:, :, :w], in1=v[:, :, w:2 * w]
                        )
                    nc.scalar.mul(out=v[:, :, :c], in_=v[:, :, :c], mul=1.0 / n_points)
                    for blk in range(sup):
                        nc.gpsimd.indirect_dma_start(
                            out=out[:],
                            out_offset=bass.IndirectOffsetOnAxis(
                                ap=offs_sb[:, t * sup + blk:t * sup + blk + 1], axis=0
                            ),
                            in_=sb[:, blk * HD:blk * HD + c],
                            in_offset=None,
                        )
            elif mode == "banks":
                # sup-grouped loads + DVE, per-block scatters rotating over
                # nbanks independent output tensors (host sums the banks;
                # row support is disjoint since destination cells are unique)
                assert T % sup == 0
                for t in range(T // sup):
                    rows = slice(t * sup * P, (t + 1) * sup * P)
                    sb = io_pool.tile([P, sup * D], mybir.dt.float32, tag="sb")
                    v = sb[:].rearrange("p (blk w) -> p blk w", w=D)
                    nc.sync.dma_start(
                        out=v,
                        in_=pf[rows, :].rearrange("(blk p) w -> p blk w", p=P),
                    )
                    w = D
                    while w > c:
                        w //= 2
                        nc.vector.tensor_add(
                            out=v[:, :, :w], in0=v[:, :, :w], in1=v[:, :, w:2 * w]
                        )
                    nc.scalar.mul(out=v[:, :, :c], in_=v[:, :, :c], mul=1.0 / n_points)
                    for blk in range(sup):
                        g = t * sup + blk
                        nc.gpsimd.indirect_dma_start(
                            out=banks[g % nbanks][:],
                            out_offset=bass.IndirectOffsetOnAxis(
                                ap=offs_sb[:, g:g + 1], axis=0
                            ),
                            in_=sb[:, blk * D:blk * D + c],
                            in_offset=None,
                        )
            elif mode == "sup":
                assert T % sup == 0
                for t in range(T // sup):
                    rows = slice(t * sup * P, (t + 1) * sup * P)
                    sb = io_pool.tile([P, sup * D], mybir.dt.float32, tag="sb")
                    v = sb[:].rearrange("p (blk w) -> p blk w", w=D)
                    nc.sync.dma_start(
                        out=v,
                        in_=pf[rows, :].rearrange("(blk p) w -> p blk w", p=P),
                    )
                    w = D
                    while w > c:
                        w //= 2
                        nc.vector.tensor_add(
                            out=v[:, :, :w], in0=v[:, :, :w], in1=v[:, :, w:2 * w]
                        )
                    nc.scalar.mul(out=v[:, :, :c], in_=v[:, :, :c], mul=1.0 / n_points)
                    for blk in range(sup):
                        nc.gpsimd.indirect_dma_start(
                            out=out[:],
                            out_offset=bass.IndirectOffsetOnAxis(
                                ap=offs_sb[:, t * sup + blk:t * sup + blk + 1], axis=0
                            ),
                            in_=sb[:, blk * D:blk * D + c],
                            in_offset=None,
                        )
            elif mode == "dve_reduce":
                for t in range(T):
                    pf_sb = io_pool.tile([P, D], mybir.dt.float32, tag="pf")
                    nc.sync.dma_start(out=pf_sb[:], in_=pf[t * P:(t + 1) * P, :])
                    w = D
                    while w > c:
                        w //= 2
                        nc.vector.tensor_add(
                            out=pf_sb[:, :w], in0=pf_sb[:, :w], in1=pf_sb[:, w:2 * w]
                        )
                    nc.scalar.mul(out=pf_sb[:, :c], in_=pf_sb[:, :c], mul=1.0 / n_points)
                    nc.gpsimd.indirect_dma_start(
                        out=out[:],
                        out_offset=bass.IndirectOffsetOnAxis(
                            ap=offs_sb[:, t:t + 1], axis=0
                        ),
                        in_=pf_sb[:, :c],
                        in_offset=None,
                    )
            else:
                raise ValueError(mode)
    nc.finalize()  # Bacc.compile(): splits multi-waits for TRN2 codegen
    return nc


def make_offs_arr(offs_r, sup, layout):
    """SBUF offsets table [128, T] matching the device pillar layout."""
    if layout == "batch":
        # pillar j = t*sup*128 + p*sup + blk  ->  offs_arr[p, t*sup + blk]
        return np.ascontiguousarray(
            offs_r.reshape(-1, P, sup).transpose(1, 0, 2).reshape(P, -1)
        )
    # pillar j = t*128 + p  ->  offs_arr[p, t]
    return np.ascontiguousarray(offs_r.reshape(-1, P).T)


def shard_inputs(point_features, voxel_coords, align=SUP * P, sup=SUP,
                 layout="batch"):
    pf = np.ascontiguousarray(
        np.asarray(point_features, dtype=np.float32).reshape(N_PILLARS, N_POINTS * C)
    )
    vc = np.asarray(voxel_coords)
    b = vc[:, 0].astype(np.int64)
    y = vc[:, 2].astype(np.int64)
    x = vc[:, 3].astype(np.int64)
    upper = (y >= HALF).astype(np.int64)
    region = b * 2 + upper
    off = (y - upper * HALF) * W + x  # row offset within the owned region
    idx_r = [np.nonzero(region == r)[0] for r in range(N_CORES)]
    nmax = max(len(ix) for ix in idx_r)
    nmax = max(align, ((nmax + align - 1) // align) * align)
    in_maps = []
    for r in range(N_CORES):
        ix = idx_r[r]
        pf_r = np.zeros((nmax, N_POINTS * C), np.float32)
        pf_r[: len(ix)] = pf[ix]
        offs_r = np.full(nmax, REGION_ROWS, np.int32)  # pad rows -> dump row
        offs_r[: len(ix)] = off[ix].astype(np.int32)
        in_maps.append({"pf": pf_r, "offs": make_offs_arr(offs_r, sup, layout)})
    return in_maps, nmax


def assemble(results):
    out = np.empty((B, C, H, W), np.float32)
    for r in range(N_CORES):
        region = results[r]["out"]
        for name, arr in results[r].items():
            if name != "out":
                region = region + arr
        o = region[:REGION_ROWS].reshape(HALF, W, C)
        b_, half = divmod(r, 2)
        out[b_, :, half * HALF:(half + 1) * HALF, :] = o.transpose(2, 0, 1)
    return out


def run(point_features, voxel_coords, trace=False, mode="banks", sup=SUP, bufs=6,
        nbanks=NBANKS, **spmd_kwargs):
    layout = "batch" if mode == "batch" else "block"
    in_maps, nmax = shard_inputs(
        point_features, voxel_coords, align=sup * P, sup=sup, layout=layout
    )
    nc = build_nc(nmax, mode=mode, sup=sup, bufs=bufs, nbanks=nbanks)
    br = run_bass_kernel_spmd(
        nc, in_maps, list(range(N_CORES)), trace=trace, **spmd_kwargs
    )
    return assemble(br.results), br


def kernel(point_features, voxel_coords):
    out, _ = run(point_features, voxel_coords)
    return out
